# revision 20
# baseline (speedup 1.0000x reference)
"""BinaryConv (binary-weight 3x3 conv) on 8 Trainium2 NeuronCores.

Full-input contract: kernel(x=[32,256,56,56] f32, weight=[256,256,3,3] f32)
-> [32,256,56,56] f32.

Strategy: data-parallel over batch (4 images/core), weight replicated.
Per core, a 1D Winograd F(4,3) decomposition ALONG H (direct taps along W):
for each H-tile of 4 output rows, 6 Winograd components l replace the 9-tap
sum with 6 comps x 3 W-taps = 18 matmul-rows per 4 output rows vs 36 direct
-- half the PE work.  Per (l, kw): out_wino[l] += Wwino[l,kw]^T . uH[l].

x ships in PHASE-MAJOR row order (host layout only): plane p holds padded
rows t=4i+p, pre-padded in H and W, so every forward-transform op is a
fully-contiguous 2D DVE op.  With Y_p = plane p rows [s:s+k], Z_p = rows
[s+1:s+1+k]:
  u0 = (Y0-Y2) + (Z0-Y2)/4      u1 = (Y3+Z0) - 4(Y1+Y2)
  u2 = 4(Y1-Y2) - (Y3-Z0)       u3 = (Z0-Y2) - 2(Y1-Y3)
  u4 = (Z0-Y2) + 2(Y1-Y3)       u5 = 4(Y1-Y3) - (Y3-Z1)
(u0 carries the G-row 1/4 factor in-data so l=0 and l=5 share an eviction
scale.)

Winograd weights are INTEGER-EXACT sign combinations (G numerators); the
fractional G scales are folded into the fp32 PSUM-eviction scales
a_g = mean|w| * [1, 1/6, 1/24] per l-pair.  PSUM l-pairs are grouped
(5,0), (1,2), (3,4): pair (5,0) needs NO weight-transform ops (raw signs),
so the PE starts real matmuls ~7us into the kernel and the HAM clock never
re-throttles; each pair shares one eviction scale, so every eviction is a
single fused ACT op.  m is stored pair-major so evictions are contiguous.

Engine balance (per image, steady state): PE ~25us of matmuls; DVE carries
all fwd/inverse tensor_tensor chains (~23us); ACT takes sign/|w|-mean, the
x4 scale prep, PSUM evictions, fwd prescales and inverse 2x/4x/8x muls
(~20us); GpSimd only issues weight/output DMA descriptors (its
tensor_tensor path is ~3.5x slower than DVE and contends for the shared
SBUF port -- measured net-negative).  Image 0 ships in two DMA chunks and
runs a split (per-hb) forward transform with hb-major matmul order so the
PE ramps immediately; x rides the sync DMA queue, weights the gpsimd
queue.

Host-side marshalling (layout/dtype/zero-pad only, all math on device):
x ships bf16 phase-major; weight ships as a tap-major bf16 transpose
[9,I,O] (sign source; sign(bf16(w)) == sign(w)) and as bf16 [O, I*9]
feeding the |w| mean.  Output ships bf16, upcast to f32 on the host.
"""

import ml_dtypes
import numpy as np

import concourse.mybir as mybir
import concourse.tile as tile
from concourse import bacc
from concourse.bass_utils import run_bass_kernel_spmd

F32 = mybir.dt.float32
BF16 = mybir.dt.bfloat16
ALU = mybir.AluOpType
ACTF = mybir.ActivationFunctionType

N_CORES = 8
B, C, H, W = 32, 256, 56, 56
O, KH, KW = 256, 3, 3
BP = B // N_CORES            # images per core
P = 128                      # partitions
NCI = C // P                 # input-channel chunks
NCO = O // P                 # output-channel chunks
NL = 6                       # Winograd F(4,3) components along H
M = 4                        # output rows per H-tile
IT = H // M                  # 14 H-tiles
IB = IT // 2                 # 7 H-tiles per psum half-block
NFREE = IB * W               # 392 <= 512 fp32 psum bank
KIN = C * KH * KW            # 2304 per-filter fan-in
W2 = W + 2                   # padded row width
NPI = IT + 1                 # rows per phase plane

# l-pair grouping per PSUM tile; eviction scale a*g per pair
PAIRS = ((5, 0), (1, 2), (3, 4))
PAIRG = (1.0, 1.0 / 6, 1.0 / 24)
# l -> (pair index, slot) in the pair-major m store
LP_OF = {5: (0, 0), 0: (0, 1), 1: (1, 0), 2: (1, 1), 3: (2, 0), 4: (2, 1)}


def build(bp: int = BP):
    nc = bacc.Bacc(
        "TRN2",
        target_bir_lowering=False,
        debug=False,
        enable_asserts=False,
        num_devices=N_CORES,
        enable_partition_id=False,
    )
    # x4[n, pc, p, i, c1, w] = xpad[n, c1*128+pc, 4i+p, w] (rows -1..58+)
    x_d = nc.dram_tensor("x4", [bp, P, M, NPI, NCI, W2], BF16,
                         kind="ExternalInput")
    wp_d = nc.dram_tensor("wp", [NCI, P, KH * KW, O], BF16,
                          kind="ExternalInput")
    wb_d = nc.dram_tensor("wb", [O, KIN], BF16, kind="ExternalInput")
    # out4[n, o, r, i, w] = out[n, o, 4i+r, w]
    out_d = nc.dram_tensor("out", [bp, O, M, IT, W], BF16,
                           kind="ExternalOutput")

    x = x_d.ap()
    wp = wp_d.ap()
    wb = wb_d.ap()
    out = out_d.ap()

    with tile.TileContext(nc) as tc:
        with (
            tc.tile_pool(name="const", bufs=1) as cpool,
            tc.tile_pool(name="wtmp", bufs=1) as wtpool,
            tc.tile_pool(name="xt", bufs=2) as xpool,
            tc.tile_pool(name="uh", bufs=2) as upool,
            tc.tile_pool(name="ft", bufs=1) as fpool,
            tc.tile_pool(name="mev", bufs=2) as mpool,
            tc.tile_pool(name="itmp", bufs=1) as ipool,
            tc.tile_pool(name="yt", bufs=2) as ypool,
            tc.tile_pool(name="psum", bufs=4, space="PSUM") as pspool,
        ):
            # ---- PE warmup: hold HAM clock through the startup ramp ------
            warm_l = cpool.tile([P, P], BF16)
            warm_r = cpool.tile([P, 512], BF16)
            nc.gpsimd.memset(warm_l[:], 0.0)
            nc.gpsimd.memset(warm_r[:], 0.0)
            zbias = cpool.tile([P, 1], F32)
            zscr = cpool.tile([P, 1], F32)
            nc.gpsimd.memset(zbias[:], 0.0)
            warm_ps = pspool.tile([P, 2, 512], F32, name="ps")
            for _ in range(24):
                nc.tensor.matmul(warm_ps[:, 0], warm_l[:], warm_r[:],
                                 start=True, stop=True)
            for _ in range(28):
                nc.tensor.matmul(warm_ps[:, 0, :128], warm_l[:],
                                 warm_r[:, :128], start=True, stop=True)
            # preload the Sign LUT on ACT before the weights arrive
            nc.scalar.sign(zscr[:], zbias[:], bias=zbias[:])

            # ---- input DMAs: x on the sync queue, weights on gpsimd ------
            def x_load(n, tile_=None, irange=None):
                xt = tile_ if tile_ is not None else \
                    xpool.tile([P, M, NPI, NCI, W2], BF16, name="xt")
                sl = slice(None) if irange is None else \
                    slice(irange[0], irange[1])
                nc.sync.dma_start(
                    xt[:, :, sl].rearrange("p m i c w -> p m (i c w)"),
                    x[n, :, :, sl].rearrange("p m i c w -> p m (i c w)"))
                return xt

            # image-0 hb0 rows ship plane-chunked (fwd0a consume order)
            xt0 = xpool.tile([P, M, NPI, NCI, W2], BF16, name="xt")
            for p in (0, 2, 1, 3):
                nc.sync.dma_start(
                    xt0[:, p, 0:8].rearrange("p i c w -> p (i c w)"),
                    x[0, :, p, 0:8].rearrange("p i c w -> p (i c w)"))
            x_load(0, tile_=xt0, irange=(8, NPI))     # rows for hb=1
            xt1 = x_load(1)

            # weights on the gpsimd DMA ring (contiguous layouts)
            wsg = cpool.tile([P, NCI, KH, KW, O], BF16, name="wsg")
            for c1 in range(NCI):
                nc.gpsimd.dma_start(
                    wsg[:, c1].rearrange("p kh kw o -> p (kh kw o)"),
                    wp[c1].rearrange("p t o -> p (t o)"))
            wstage = cpool.tile([P, KIN], BF16, name="ws")
            nc.gpsimd.dma_start(wstage[:], wb[0:P, :])

            # ---- sign in place (ACT), kh-chunked; kh 0,2 first (pair 5,0)
            for kh in (0, 2, 1):
                nc.scalar.sign(wsg[:, :, kh], wsg[:, :, kh], bias=zbias[:])

            # ---- |w| means + per-pair eviction scales a_g = mean|w|*g ----
            asums = [cpool.tile([P, 1], F32, name=f"as{co}")
                     for co in range(NCO)]
            a_g = cpool.tile([P, NCO, 3], F32, name="ag")
            for co in range(NCO):
                if co > 0:
                    nc.gpsimd.dma_start(wstage[:],
                                        wb[co * P:(co + 1) * P, :])
                nc.scalar.activation(
                    wstage[:], wstage[:], ACTF.Abs,
                    bias=zbias[:], accum_out=asums[co][:])
                for k, g in enumerate(PAIRG):
                    nc.scalar.mul(a_g[:, co, k:k + 1], asums[co][:],
                                  g / KIN)

            # ---- integer Wwino combos (exact sign sums) ------------------
            # l=0: s0 (raw)      l=5: s2 (raw)
            # W1 = -(s0+s1+s2)   W2 = s1-s0-s2      (pair scale a/6)
            # W3 = s0+2s1+4s2    W4 = s0-2s1+4s2    (pair scale a/24)
            wt = cpool.tile([P, NCI, 4, KW, O], BF16, name="wt")
            s12t = [wtpool.tile([P, NCI, O], BF16, name=f"s12_{kw}")
                    for kw in range(KW)]

            def s_(kh, kw):
                return wsg[:, :, kh, kw]      # [P, NCI, O]

            def combos12():
                for kw in range(KW):
                    e1 = wtpool.tile([P, NCI, O], BF16, name=f"we{kw}")
                    nc.vector.tensor_tensor(
                        e1[:], s_(0, kw), s_(2, kw), op=ALU.add)
                    nc.vector.tensor_tensor(
                        wt[:, :, 1, kw], s_(1, kw), e1[:], op=ALU.subtract)
                    nc.vector.tensor_scalar_mul(
                        s12t[kw][:], s_(1, kw), 2.0)
                    # W1 = W2 - 2*s1 = -(s0+s1+s2)
                    nc.vector.tensor_tensor(
                        wt[:, :, 0, kw], wt[:, :, 1, kw], s12t[kw][:],
                        op=ALU.subtract)

            def combos34():
                for kw in range(KW):
                    t34 = wtpool.tile([P, NCI, O], BF16, name=f"wt{kw}")
                    nc.vector.tensor_scalar_mul(t34[:], s_(2, kw), 4.0)
                    nc.vector.tensor_tensor(
                        t34[:], t34[:], s_(0, kw), op=ALU.add)
                    nc.vector.tensor_tensor(
                        wt[:, :, 2, kw], t34[:], s12t[kw][:], op=ALU.add)
                    nc.vector.tensor_tensor(
                        wt[:, :, 3, kw], t34[:], s12t[kw][:],
                        op=ALU.subtract)

            def lhsT(c1, l, kw, co):
                if l == 0:
                    return wsg[:, c1, 0, kw, co * P:(co + 1) * P]
                if l == 5:
                    return wsg[:, c1, 2, kw, co * P:(co + 1) * P]
                return wt[:, c1, l - 1, kw, co * P:(co + 1) * P]

            # ---- forward transform (see module docstring) ----------------
            # Emits u in pair-consumption order u0,u5,u1,u2,u3,u4.
            # Scale-by-constant prescales ride the ACT engine.
            def fwd(xt, uh, rng=(0, IT), interleave=None, dve_ts=False):
                i0, i1 = rng
                k = i1 - i0
                sz = k * NCI * W2
                ft = [fpool.tile([P, IT * NCI * W2], BF16, name=f"f{j}")
                      for j in range(5)]
                fA, fB, fC, fD, fE = [t[:, :sz] for t in ft]

                def pl(p, s):     # plane p rows [s : s+k], flat [P, sz]
                    return xt[:, p, s:s + k].rearrange(
                        "p i c w -> p (i c w)")

                def u(l):
                    return uh[:, l, i0:i1].rearrange("p i c w -> p (i c w)")

                Y0, Y1, Y2, Y3 = pl(0, i0), pl(1, i0), pl(2, i0), pl(3, i0)
                Z0, Z1 = pl(0, i0 + 1), pl(1, i0 + 1)
                tt = nc.vector.tensor_tensor
                sm = nc.vector.tensor_scalar_mul if dve_ts else \
                    nc.scalar.mul
                il = interleave if interleave else (lambda j: None)
                tt(fC, Y0, Y2, op=ALU.subtract)         # w1
                tt(fD, Z0, Y2, op=ALU.subtract)         # tc
                sm(fE, fD, 0.25)                        # tcq (ACT)
                tt(u(0), fC, fE, op=ALU.add)            # u0 = w1 + tc/4
                il(0)
                tt(fA, Y1, Y3, op=ALU.subtract)         # tp
                sm(fC, fA, 4.0)                         # tp4 (ACT; w1 dead)
                tt(fB, Y3, Z1, op=ALU.subtract)         # q5
                tt(u(5), fC, fB, op=ALU.subtract)       # u5 = 4*tp - q5
                il(1)
                tt(fE, Y1, Y2, op=ALU.add)              # q1 (tcq dead)
                sm(fC, fE, 4.0)                         # q14 (ACT)
                tt(fB, Y3, Z0, op=ALU.add)              # q2 (q5 dead)
                tt(u(1), fB, fC, op=ALU.subtract)       # u1 = q2 - 4*q1
                il(2)
                tt(fE, Y1, Y2, op=ALU.subtract)         # r1 (q1 dead)
                sm(fC, fE, 4.0)                         # r14 (ACT)
                tt(fB, Y3, Z0, op=ALU.subtract)         # r2 (q2 dead)
                tt(u(2), fC, fB, op=ALU.subtract)       # u2 = 4*r1 - r2
                il(3)
                sm(fE, fA, 2.0)                         # tp2 (ACT; r1 dead)
                tt(u(3), fD, fE, op=ALU.subtract)       # u3 = tc - 2*tp
                tt(u(4), fD, fE, op=ALU.add)            # u4 = tc + 2*tp
                il(4)

            # ---- inverse transform y = A^T m -----------------------------
            # DVE tensor_tensor chain; 2x/4x/8x scale muls on ACT.
            def inverse(m, y, cs, hs=slice(None), dve_ts=False,
                        fr=None):
                ncs = NCO if cs == slice(None) else 1
                nhs = 2 if hs == slice(None) else 1
                f0, f1 = fr if fr else (0, ncs * nhs * NFREE)
                sz = f1 - f0

                def mv(l):
                    pi, j = LP_OF[l]
                    return m[:, pi, j, cs, hs].rearrange(
                        "p c h w -> p (c h w)")[:, f0:f1]

                tiles = [ipool.tile([P, NCO, 2, NFREE], BF16, name=nm)
                         for nm in ("e", "o", "f", "g", "t")]
                ev, ov, fv, gv, tv = [
                    q[:].rearrange("p c h w -> p (c h w)")[:, f0:f1]
                    for q in tiles]
                hsl = slice(None) if hs == slice(None) else \
                    slice(hs.start * IB, hs.start * IB + IB)
                dtt = nc.vector.tensor_tensor
                amul = (lambda o_, i_, s_:
                        nc.vector.tensor_scalar_mul(o_, i_, s_)) \
                    if dve_ts else nc.scalar.mul

                def yv(r):
                    return y[:, r, cs, hsl].rearrange(
                        "p c h w -> p (c h w)")[:, f0:f1]

                dtt(ev, mv(1), mv(2), op=ALU.add)
                dtt(ov, mv(1), mv(2), op=ALU.subtract)
                dtt(fv, mv(3), mv(4), op=ALU.add)
                dtt(gv, mv(3), mv(4), op=ALU.subtract)
                dtt(tv, mv(0), ev, op=ALU.add)
                dtt(yv(0), tv, fv, op=ALU.add)
                amul(tv, fv, 4.0)                        # f4 (t dead)
                dtt(yv(2), tv, ev, op=ALU.add)           # y2 = 4*f + e
                amul(ev, gv, 2.0)                        # g2 (e dead)
                dtt(yv(1), ev, ov, op=ALU.add)           # y1 = 2*g + o
                dtt(tv, mv(5), ov, op=ALU.add)           # t2 (f4 dead)
                amul(fv, gv, 8.0)                        # g8 (f dead)
                dtt(yv(3), fv, tv, op=ALU.add)           # y3 = 8*g + t2

            # ---- matmul group for one (co, hb, pair) ---------------------
            def mm_group(uh, m, co, hb, pi):
                la, lb = PAIRS[pi]
                i0 = hb * IB
                ps = pspool.tile([P, 2, 512], F32, name="ps")
                for j, l in enumerate((la, lb)):
                    for c1 in range(NCI):
                        for kw in range(KW):
                            nc.tensor.matmul(
                                ps[:, j, :NFREE],
                                lhsT(c1, l, kw, co),
                                uh[:, l, i0:i0 + IB, c1, kw:kw + W],
                                start=(c1 == 0 and kw == 0),
                                stop=(c1 == NCI - 1 and kw == KW - 1),
                            )
                nc.scalar.mul(m[:, pi, :, co, hb], ps[:, :, :NFREE],
                              a_g[:, co, pi:pi + 1])

            # ---- image 0: split fwd, hb-major matmuls --------------------
            uh0 = upool.tile([P, NL, IT, NCI, W2], BF16, name="uh")
            fwd(xt0, uh0, rng=(0, IB), dve_ts=True,
                interleave=lambda j: combos12() if j == 2 else None)
            combos34()
            fwd(xt0, uh0, rng=(IB, IT), dve_ts=True)

            m0 = mpool.tile([P, 3, 2, NCO, 2, NFREE], BF16, name="m")
            y0 = ypool.tile([P, M, NCO, IT, W], BF16, name="y")
            for hb in range(2):
                for pi in range(3):
                    for co in range(NCO):
                        mm_group(uh0, m0, co, hb, pi)
            uh1 = upool.tile([P, NL, IT, NCI, W2], BF16, name="uh")
            fwd(xt1, uh1)

            # ---- images 1..bp-1 ------------------------------------------
            uhs, nxt_uh = uh0, uh1
            mt, y = m0, y0
            xts = [None, xt1]
            for n in range(1, bp):
                last = n + 1 == bp
                if not last:
                    xts.append(x_load(n + 1))
                prev_m, prev_y = mt, y
                uhs = nxt_uh
                mt = mpool.tile([P, 3, 2, NCO, 2, NFREE], BF16, name="m")
                y = ypool.tile([P, M, NCO, IT, W], BF16, name="y")
                if last:
                    # emit the previous image's inverse FIRST so it runs
                    # during (not after) the last image's matmul phase
                    inverse(prev_m, prev_y, slice(None))
                    for co in range(NCO):
                        nc.sync.dma_start(
                            out[n - 1, co * P:(co + 1) * P, :, :, :],
                            prev_y[:, :, co])
                for co in range(NCO):
                    for hb in range(2):
                        for pi in range(3):
                            mm_group(uhs, mt, co, hb, pi)
                        if last:
                            final = co == NCO - 1 and hb == 1
                            if not final:
                                inverse(mt, y, slice(co, co + 1),
                                        slice(hb, hb + 1), dve_ts=True)
                                nc.gpsimd.dma_start(
                                    out[n, co * P:(co + 1) * P, :,
                                        hb * IB:(hb + 1) * IB, :],
                                    y[:, :, co, hb * IB:(hb + 1) * IB])
                            else:
                                # split the very last quarter in two so
                                # the tail inverse+DMA pipeline
                                isp = 3           # i-rows 0:3 then 3:7
                                for ia, ib in ((0, isp), (isp, IB)):
                                    inverse(mt, y, slice(co, co + 1),
                                            slice(hb, hb + 1),
                                            dve_ts=True,
                                            fr=(ia * W, ib * W))
                                    nc.gpsimd.dma_start(
                                        out[n, co * P:(co + 1) * P, :,
                                            hb * IB + ia:hb * IB + ib,
                                            :],
                                        y[:, :, co,
                                          hb * IB + ia:hb * IB + ib])
                if not last:
                    # fwd for the next image AFTER all of this image's
                    # groups: its ACT prescales must not block evictions
                    nxt_uh = upool.tile([P, NL, IT, NCI, W2], BF16,
                                        name="uh")
                    fwd(xts[n + 1], nxt_uh)
                    # inverse + store for the PREVIOUS image (m complete)
                    inverse(prev_m, prev_y, slice(None))
                    for co in range(NCO):
                        nc.sync.dma_start(
                            out[n - 1, co * P:(co + 1) * P, :, :, :],
                            prev_y[:, :, co])

    nc.compile()
    return nc


_NC_CACHE: dict[int, object] = {}


def _get_nc(bp: int = BP):
    if bp not in _NC_CACHE:
        _NC_CACHE[bp] = build(bp)
    return _NC_CACHE[bp]


def make_in_maps(x: np.ndarray, weight: np.ndarray, n_cores: int = N_CORES,
                 bp: int = BP):
    x = np.ascontiguousarray(x, dtype=np.float32)
    weight = np.ascontiguousarray(weight, dtype=np.float32)
    xb = x.astype(ml_dtypes.bfloat16)
    # phase-major padded layout: x4[n, pc, p, i, c1, w2]
    xpad = np.zeros((B, C, M * NPI, W2), dtype=ml_dtypes.bfloat16)
    xpad[:, :, 1:H + 1, 1:W + 1] = xb
    x4 = np.ascontiguousarray(
        xpad.reshape(B, NCI, P, NPI, M, W2).transpose(0, 2, 4, 3, 1, 5))
    wp = np.ascontiguousarray(
        weight.reshape(O, C, KH * KW).transpose(1, 2, 0).reshape(
            NCI, P, KH * KW, O)
    ).astype(ml_dtypes.bfloat16)  # [c1, p, t, o]
    wb = weight.reshape(O, KIN).astype(ml_dtypes.bfloat16)
    return [
        {"x4": x4[i * bp:(i + 1) * bp], "wp": wp, "wb": wb}
        for i in range(n_cores)
    ]


def kernel(x: np.ndarray, weight: np.ndarray) -> np.ndarray:
    nc = _get_nc(BP)
    in_maps = make_in_maps(x, weight)
    res = run_bass_kernel_spmd(nc, in_maps, core_ids=list(range(N_CORES)))
    out = np.empty((B, O, H, W), dtype=np.float32)
    for i in range(N_CORES):
        o4 = res.results[i]["out"].astype(np.float32)      # [bp,O,4,14,56]
        out[i * BP:(i + 1) * BP] = (
            o4.reshape(BP, O, M, IT, W).transpose(0, 1, 3, 2, 4)
            .reshape(BP, O, H, W))
    return out
